# revision 2
# baseline (speedup 1.0000x reference)
"""Trainium2 Bass kernel for nn_NeuronPool (MoE-style neuron pool routing).

Problem: 2048 tokens (4x512) each select K=4 of 512 "neurons" (experts);
per (token, selection): out = gelu(x @ w_in[e] + b_in[e]) @ w_out[e] + b_out[e]
with DM=256, DF=64.

Strategy (expert-parallel over 8 NeuronCores):
- Host: assign 64 experts per core (snake deal by routed-pair count so the
  per-rank count profiles match across cores), sort each core's experts by
  count, and bake a shared per-rank capacity CAP_j = max over cores of the
  rank-j count. One SPMD program serves all 8 cores; each core gets its own
  packed inputs. Host gathers/transposes x into per-slot contiguous columns
  (the "token dispatch by neuron id" of the sharding hint) and un-permutes
  the output.
- Device (per core): stream w_in/w_out (f32, 1 MiB DMA groups) and the
  dispatched xT through the PE:
    mm1: hiddenT[64, t] = w_in[e]^T @ xT_cols   (2 accumulated K=128 chunks;
         even slots -> PE col-group 0, odd -> col-group 64)
    ACT: exact-erf GELU with fused per-partition bias_in, PSUM -> SBUF
    mm2: outT[128, t] = w_out[e][:, chunk]^T @ hiddenT  (2 chunks; even slots
         use PE row-group 0, odd row-group 64 - packed in one 128-partition
         W2 tile so weight DMAs use all partitions)
    DVE: evacuate PSUM outT into a [128, 2, NPAD] transposed output buffer
    DMA out per group.
All matmul moving sizes are t (~8-40), so fp32's 4 cycles/row is cheap; the
kernel is HBM-bound on the 8 MiB/core of expert weights, which are read
exactly once - the memory roofline for this problem.
"""

import os
import numpy as np

import concourse.bass as bass
import concourse.mybir as mybir
import concourse.bacc as bacc
import concourse.tile as tile
from concourse.bass_utils import run_bass_kernel_spmd

POOL, DM, DF = 512, 256, 64
B, S, K = 4, 512, 4
NT = B * S
NPAIR = NT * K
NCORE = 8
EPC = POOL // NCORE  # experts per core
GS = 8  # slots per DMA/compute group

# Set by run when KERNEL_TRACE=1: exec time of the slowest core, ns.
LAST_EXEC_TIME_NS = None
LAST_TRACE_PATH = None

_CACHE: dict = {}


# ----------------------------------------------------------------- layout --


class _Layout:
    pass


def _build_layout(e_flat: np.ndarray) -> "_Layout":
    """Shared (cross-core) slot structure + per-core expert/pair assignment."""
    lay = _Layout()
    counts = np.bincount(e_flat, minlength=POOL)
    order = np.argsort(-counts, kind="stable")

    # snake deal: ranks 0..15 -> cores 0..7,7..0, repeating
    core_expert = [[] for _ in range(NCORE)]
    for r, ex in enumerate(order):
        q, rr = divmod(r, NCORE)
        c = rr if q % 2 == 0 else NCORE - 1 - rr
        core_expert[c].append(int(ex))
    # per-core lists are count-descending by construction

    # shared capacity per rank
    rank_cap = [
        max(int(counts[core_expert[c][j]]) for c in range(NCORE)) for j in range(EPC)
    ]

    # slots: (rank, chunk, cap) with cap <= 128; drop empty ranks
    slots = []
    for j in range(EPC):
        cap = rank_cap[j]
        k = 0
        while cap > 0:
            c_ = min(cap, 128)
            slots.append((j, k, c_))
            cap -= c_
            k += 1
    if len(slots) % 2 == 1:
        slots.append((-1, 0, 2))  # dummy slot (zero weights/x, output discarded)

    lay.slots = slots
    lay.nslot = len(slots)
    caps = np.array([s[2] for s in slots], dtype=np.int64)
    lay.caps = caps
    lay.offs = np.concatenate([[0], np.cumsum(caps)])[:-1]
    lay.npad = int(caps.sum())
    lay.core_expert = core_expert

    # per-core, per-slot: expert id and actual pair count
    sorted_pairs = np.argsort(e_flat, kind="stable")
    starts = np.concatenate([[0], np.cumsum(counts)])
    lay.slot_expert = np.zeros((NCORE, lay.nslot), dtype=np.int64)
    lay.slot_pairs = []  # [core][slot] -> pair ids (np array, len<=cap)
    for c in range(NCORE):
        plist = []
        for (j, k, cap) in slots:
            if j < 0:
                lay.slot_expert[c][len(plist)] = 0  # dummy; weights zeroed
                plist.append(np.zeros((0,), dtype=np.int64))
                continue
            ex = core_expert[c][j]
            lay.slot_expert[c][len(plist)] = ex
            pp = sorted_pairs[starts[ex] : starts[ex] + counts[ex]]
            pp = pp[k * 128 : k * 128 + cap]
            plist.append(pp)
        lay.slot_pairs.append(plist)
    return lay


def _prep_core_inputs(lay, c, x_flat, w_in, w_out, bias_in, bias_out, has_bout):
    ns, npad = lay.nslot, lay.npad
    ex = lay.slot_expert[c]  # [ns]
    dummy = np.array([s[0] < 0 for s in lay.slots])

    xp = np.zeros((npad, DM), dtype=np.float32)
    for si in range(ns):
        pp = lay.slot_pairs[c][si]
        if len(pp):
            o = lay.offs[si]
            xp[o : o + len(pp)] = x_flat[pp // K]
    XT = np.ascontiguousarray(xp.T)  # [256, npad]

    w1 = w_in[ex].astype(np.float32).copy()  # [ns, 256, 64]
    w2 = w_out[ex].astype(np.float32).copy()  # [ns, 64, 256]
    bi = bias_in[ex].astype(np.float32).copy()  # [ns, 64]
    if dummy.any():
        w1[dummy] = 0.0
        w2[dummy] = 0.0
        bi[dummy] = 0.0
    W1 = np.ascontiguousarray(
        w1.reshape(ns, 2, 128, DF).transpose(2, 0, 1, 3)
    )  # [128, ns, 2, 64]
    W2 = np.ascontiguousarray(
        w2.reshape(ns // 2, 2, DF, 2, 128).transpose(1, 2, 0, 3, 4).reshape(
            128, ns // 2, 2, 128
        )
    )  # [128, ns/2, 2, 128]
    BIN = np.ascontiguousarray(
        bi.reshape(ns // 2, 2, DF).transpose(1, 2, 0).reshape(128, ns // 2)
    )  # [128, ns/2]
    m = {"XT": XT, "W1": W1, "W2": W2, "BIN": BIN}
    if has_bout:
        bo = bias_out[ex].astype(np.float32).copy()
        if dummy.any():
            bo[dummy] = 0.0
        m["BOUT"] = np.ascontiguousarray(bo.reshape(ns, 2, 128).transpose(2, 1, 0))
    return m


# ----------------------------------------------------------------- device --


def _build_program(lay, has_bout):
    ns, npad = lay.nslot, lay.npad
    caps, offs = lay.caps, lay.offs
    capmax = int(caps.max())
    ngrp = (ns + GS - 1) // GS
    f32 = mybir.dt.float32

    nc = bacc.Bacc("TRN2", target_bir_lowering=False, debug=False, num_devices=NCORE)
    dXT = nc.dram_tensor("XT", [DM, npad], f32, kind="ExternalInput").ap()
    dW1 = nc.dram_tensor("W1", [128, ns, 2, DF], f32, kind="ExternalInput").ap()
    dW2 = nc.dram_tensor("W2", [128, ns // 2, 2, 128], f32, kind="ExternalInput").ap()
    dBIN = nc.dram_tensor("BIN", [128, ns // 2], f32, kind="ExternalInput").ap()
    if has_bout:
        dBOUT = nc.dram_tensor("BOUT", [128, 2, ns], f32, kind="ExternalInput").ap()
    dOUT = nc.dram_tensor("OUTT", [128, 2, npad], f32, kind="ExternalOutput").ap()

    gelu = mybir.ActivationFunctionType.Gelu

    with tile.TileContext(nc) as tc:
        with (
            tc.tile_pool(name="wpool", bufs=3) as wpool,
            tc.tile_pool(name="apool", bufs=3) as apool,
            tc.tile_pool(name="cpool", bufs=1) as cpool,
            tc.tile_pool(name="hpool", bufs=3) as hpool,
            tc.tile_pool(name="opool", bufs=3) as opool,
            tc.tile_pool(name="ps1", bufs=2, space="PSUM") as ps1pool,
            tc.tile_pool(name="ps2", bufs=2, space="PSUM") as ps2pool,
        ):
            tBIN = cpool.tile([128, ns // 2], f32, tag="bin")
            nc.sync.dma_start(out=tBIN[:], in_=dBIN[:])
            if has_bout:
                tBOUT = cpool.tile([128, 2, ns], f32, tag="bout")
                nc.sync.dma_start(out=tBOUT[:], in_=dBOUT[:])

            for g in range(ngrp):
                s0 = g * GS
                s1 = min(ns, s0 + GS)
                gsl = s1 - s0  # slots in group (even)
                goff = int(offs[s0])
                gw = int(offs[s1 - 1] + caps[s1 - 1]) - goff

                tW1 = wpool.tile([128, gsl, 2, DF], f32, tag="w1")
                nc.sync.dma_start(out=tW1[:], in_=dW1[:, s0:s1, :, :])
                tW2 = wpool.tile([128, gsl // 2, 2, 128], f32, tag="w2")
                nc.sync.dma_start(out=tW2[:], in_=dW2[:, s0 // 2 : s1 // 2, :, :])
                tA0 = apool.tile([128, gw], f32, tag="a0")
                nc.sync.dma_start(out=tA0[:], in_=dXT[0:128, goff : goff + gw])
                tA1 = apool.tile([128, gw], f32, tag="a1")
                nc.sync.dma_start(out=tA1[:], in_=dXT[128:256, goff : goff + gw])

                tO = opool.tile([128, 2, gw], f32, tag="ot")

                for p in range(gsl // 2):
                    se, so = s0 + 2 * p, s0 + 2 * p + 1
                    nE, nO = int(caps[se]), int(caps[so])
                    oE = int(offs[se]) - goff
                    oO = int(offs[so]) - goff

                    tP1e = ps1pool.tile([128, capmax], f32, tag="ps1e")
                    tP1o = ps1pool.tile([128, capmax], f32, tag="ps1o")
                    tH = hpool.tile([128, capmax], f32, tag="h")

                    for ch, tA in enumerate((tA0, tA1)):
                        nc.tensor.matmul(
                            tP1e[0:DF, 0:nE],
                            lhsT=tW1[:, 2 * p, ch, :],
                            rhs=tA[:, oE : oE + nE],
                            start=(ch == 0),
                            stop=(ch == 1),
                        )
                    for ch, tA in enumerate((tA0, tA1)):
                        nc.tensor.matmul(
                            tP1o[DF:128, 0:nO],
                            lhsT=tW1[:, 2 * p + 1, ch, :],
                            rhs=tA[:, oO : oO + nO],
                            start=(ch == 0),
                            stop=(ch == 1),
                        )

                    jj = (se // 2) % (GS // 2)
                    nc.scalar.activation(
                        tH[0:DF, 0:nE],
                        tP1e[0:DF, 0:nE],
                        gelu,
                        bias=tBIN[0:DF, se // 2 : se // 2 + 1],
                    )
                    nc.scalar.activation(
                        tH[DF:128, 0:nO],
                        tP1o[DF:128, 0:nO],
                        gelu,
                        bias=tBIN[DF:128, se // 2 : se // 2 + 1],
                    )

                    tP2e = ps2pool.tile([128, 2, capmax], f32, tag="ps2e")
                    tP2o = ps2pool.tile([128, 2, capmax], f32, tag="ps2o")
                    for ch in range(2):
                        nc.tensor.matmul(
                            tP2e[:, ch, 0:nE],
                            lhsT=tW2[0:DF, jj, ch, :],
                            rhs=tH[0:DF, 0:nE],
                            start=True,
                            stop=True,
                        )
                    for ch in range(2):
                        nc.tensor.matmul(
                            tP2o[:, ch, 0:nO],
                            lhsT=tW2[DF:128, jj, ch, :],
                            rhs=tH[DF:128, 0:nO],
                            start=True,
                            stop=True,
                        )

                    if has_bout:
                        for ch in range(2):
                            nc.scalar.activation(
                                tO[:, ch, oE : oE + nE],
                                tP2e[:, ch, 0:nE],
                                mybir.ActivationFunctionType.Identity,
                                bias=tBOUT[:, ch, se : se + 1],
                            )
                            nc.scalar.activation(
                                tO[:, ch, oO : oO + nO],
                                tP2o[:, ch, 0:nO],
                                mybir.ActivationFunctionType.Identity,
                                bias=tBOUT[:, ch, so : so + 1],
                            )
                    else:
                        nc.vector.tensor_copy(tO[:, :, oE : oE + nE], tP2e[:, :, 0:nE])
                        nc.vector.tensor_copy(tO[:, :, oO : oO + nO], tP2o[:, :, 0:nO])

                nc.sync.dma_start(out=dOUT[:, :, goff : goff + gw], in_=tO[:])

    nc.compile()
    return nc


# ----------------------------------------------------------------- driver --


def _install_profile_shim():
    """antenv.axon_hooks shim: NTFF profiling via libaxon_pjrt exports."""
    import contextlib
    import ctypes
    import sys
    import types

    if "antenv.axon_hooks" in sys.modules:
        return

    lib = ctypes.CDLL("/opt/axon/libaxon_pjrt.so")
    for fn in (lib.axon_start_nrt_profile, lib.axon_stop_nrt_profile):
        fn.restype = ctypes.c_int64
        fn.argtypes = [ctypes.c_char_p, ctypes.c_size_t]

    @contextlib.contextmanager
    def _ntff_profile(output_dir: str, device_ids):
        import jax

        jax.devices()
        d = output_dir.encode()
        rc = lib.axon_start_nrt_profile(d, len(d))
        assert rc == 0, f"axon_start_nrt_profile rc={rc}"
        try:
            yield
        finally:
            lib.axon_stop_nrt_profile(d, len(d))

    mod = types.ModuleType("antenv.axon_hooks")
    mod.get_axon_ntff_profile_hook = lambda: _ntff_profile
    sys.modules["antenv.axon_hooks"] = mod
    import concourse.bass_utils as bu

    bu.upload_artifacts = lambda tmpdir: f"local:{tmpdir}"


def kernel(x, indices, w_in, w_out, bias_in, bias_out):
    global LAST_EXEC_TIME_NS, LAST_TRACE_PATH
    x = np.asarray(x, dtype=np.float32)
    w_in = np.asarray(w_in, dtype=np.float32)
    w_out = np.asarray(w_out, dtype=np.float32)
    bias_in = np.asarray(bias_in, dtype=np.float32)
    bias_out = np.asarray(bias_out, dtype=np.float32)
    e_flat = np.asarray(indices).reshape(-1).astype(np.int64)
    assert x.shape == (B, S, DM) and e_flat.shape == (NPAIR,)

    has_bout = bool(np.any(bias_out))
    lay = _build_layout(e_flat)
    key = (lay.nslot, tuple(int(c) for c in lay.caps), has_bout)
    if key in _CACHE:
        nc = _CACHE[key]
    else:
        nc = _build_program(lay, has_bout)
        _CACHE[key] = nc

    x_flat = x.reshape(NT, DM)
    in_maps = [
        _prep_core_inputs(lay, c, x_flat, w_in, w_out, bias_in, bias_out, has_bout)
        for c in range(NCORE)
    ]

    trace = os.environ.get("KERNEL_TRACE", "") == "1"
    if trace:
        _install_profile_shim()
        res = run_bass_kernel_spmd(
            nc, in_maps, list(range(NCORE)), trace=True,
            trace_cores=[0], stitch_traces=False,
        )
        LAST_EXEC_TIME_NS = res.exec_time_ns
        it = res.instructions_and_trace
        LAST_TRACE_PATH = it[1] if it else None
    else:
        res = run_bass_kernel_spmd(nc, in_maps, list(range(NCORE)))

    out_flat = np.zeros((NPAIR, DM), dtype=np.float32)
    for c in range(NCORE):
        OUTT = res.results[c]["OUTT"]  # [128, 2, npad]
        rows = OUTT.transpose(2, 1, 0).reshape(lay.npad, DM)
        for si in range(lay.nslot):
            pp = lay.slot_pairs[c][si]
            if len(pp):
                o = lay.offs[si]
                out_flat[pp] = rows[o : o + len(pp)]
    return out_flat.reshape(B, S, K, DM)


# revision 6
# speedup vs baseline: 1.3106x; 1.3106x over previous
"""Trainium2 Bass kernel for nn_NeuronPool (MoE-style neuron pool routing).

Problem: 2048 tokens (4x512) each select K=4 of 512 "neurons" (experts);
per (token, selection): out = gelu(x @ w_in[e] + b_in[e]) @ w_out[e] + b_out[e]
with DM=256, DF=64.

Strategy (expert-parallel over 8 NeuronCores):
- Host: assign 64 experts per core (snake deal by routed-pair count so the
  per-rank count profiles match across cores), sort each core's experts by
  count, and bake a shared per-rank capacity CAP_j = max over cores of the
  rank-j count. One SPMD program serves all 8 cores; each core gets its own
  packed inputs. Host gathers/transposes x into per-slot contiguous columns
  (the "token dispatch by neuron id" of the sharding hint) and un-permutes
  the output.
- Device (per core): stream w_in/w_out (f32, 1 MiB DMA groups) and the
  dispatched xT through the PE:
    mm1: hiddenT[64, t] = w_in[e]^T @ xT_cols   (2 accumulated K=128 chunks;
         even slots -> PE col-group 0, odd -> col-group 64)
    ACT: exact-erf GELU with fused per-partition bias_in, PSUM -> SBUF
    mm2: outT[128, t] = w_out[e][:, chunk]^T @ hiddenT  (2 chunks; even slots
         use PE row-group 0, odd row-group 64 - packed in one 128-partition
         W2 tile so weight DMAs use all partitions)
    DVE: evacuate PSUM outT into a [128, 2, NPAD] transposed output buffer
    DMA out per group.
All matmul moving sizes are t (~8-40), so fp32's 4 cycles/row is cheap; the
kernel is HBM-bound on the 8 MiB/core of expert weights, which are read
exactly once - the memory roofline for this problem.
"""

import os
import numpy as np

import concourse.bass as bass
import concourse.mybir as mybir
import concourse.bacc as bacc
import concourse.tile as tile
from concourse.bass_utils import run_bass_kernel_spmd

POOL, DM, DF = 512, 256, 64
B, S, K = 4, 512, 4
NT = B * S
NPAIR = NT * K
NCORE = 8
EPC = POOL // NCORE  # experts per core
GS = 8  # slots per DMA/compute group

# Set by run when KERNEL_TRACE=1: exec time of the slowest core, ns.
LAST_EXEC_TIME_NS = None
LAST_TRACE_PATH = None

_CACHE: dict = {}


# ----------------------------------------------------------------- layout --


class _Layout:
    pass


def _build_layout(e_flat: np.ndarray) -> "_Layout":
    """Shared (cross-core) slot structure + per-core expert/pair assignment."""
    lay = _Layout()
    counts = np.bincount(e_flat, minlength=POOL)
    order = np.argsort(-counts, kind="stable")

    # snake deal: ranks 0..15 -> cores 0..7,7..0, repeating
    core_expert = [[] for _ in range(NCORE)]
    for r, ex in enumerate(order):
        q, rr = divmod(r, NCORE)
        c = rr if q % 2 == 0 else NCORE - 1 - rr
        core_expert[c].append(int(ex))
    # per-core lists are count-descending by construction

    # shared capacity per rank
    rank_cap = [
        max(int(counts[core_expert[c][j]]) for c in range(NCORE)) for j in range(EPC)
    ]

    # slots: (rank, chunk, cap) with cap <= 128; drop empty ranks
    slots = []
    for j in range(EPC):
        cap = rank_cap[j]
        k = 0
        while cap > 0:
            c_ = min(cap, 128)
            slots.append((j, k, c_))
            cap -= c_
            k += 1
    if len(slots) % 2 == 1:
        slots.append((-1, 0, 2))  # dummy slot (zero weights/x, output discarded)

    lay.slots = slots
    lay.nslot = len(slots)
    caps = np.array([s[2] for s in slots], dtype=np.int64)
    lay.caps = caps
    lay.offs = np.concatenate([[0], np.cumsum(caps)])[:-1]
    lay.npad = int(caps.sum())
    lay.core_expert = core_expert

    # per-core, per-slot: expert id and actual pair count
    sorted_pairs = np.argsort(e_flat, kind="stable")
    starts = np.concatenate([[0], np.cumsum(counts)])
    lay.slot_expert = np.zeros((NCORE, lay.nslot), dtype=np.int64)
    lay.slot_pairs = []  # [core][slot] -> pair ids (np array, len<=cap)
    for c in range(NCORE):
        plist = []
        for (j, k, cap) in slots:
            if j < 0:
                lay.slot_expert[c][len(plist)] = 0  # dummy; weights zeroed
                plist.append(np.zeros((0,), dtype=np.int64))
                continue
            ex = core_expert[c][j]
            lay.slot_expert[c][len(plist)] = ex
            pp = sorted_pairs[starts[ex] : starts[ex] + counts[ex]]
            pp = pp[k * 128 : k * 128 + cap]
            plist.append(pp)
        lay.slot_pairs.append(plist)
    return lay


def _prep_core_inputs(lay, c, x_flat, w_in, w_out, bias_in, bias_out, has_bout):
    ns, npad = lay.nslot, lay.npad
    ex = lay.slot_expert[c]  # [ns]
    dummy = np.array([s[0] < 0 for s in lay.slots])

    xp = np.zeros((npad, DM), dtype=np.float32)
    for si in range(ns):
        pp = lay.slot_pairs[c][si]
        if len(pp):
            o = lay.offs[si]
            xp[o : o + len(pp)] = x_flat[pp // K]
    XT = np.ascontiguousarray(xp.T)  # [256, npad]

    w1 = w_in[ex].astype(np.float32).copy()  # [ns, 256, 64]
    w2 = w_out[ex].astype(np.float32).copy()  # [ns, 64, 256]
    bi = bias_in[ex].astype(np.float32).copy()  # [ns, 64]
    if dummy.any():
        w1[dummy] = 0.0
        w2[dummy] = 0.0
        bi[dummy] = 0.0
    W1 = np.ascontiguousarray(
        w1.reshape(ns, 2, 128, DF).transpose(2, 0, 1, 3)
    )  # [128, ns, 2, 64]
    W2 = np.ascontiguousarray(
        w2.reshape(ns // 2, 2, DF, 2, 128).transpose(1, 2, 0, 3, 4).reshape(
            128, ns // 2, 2, 128
        )
    )  # [128, ns/2, 2, 128]
    BIN = np.ascontiguousarray(
        bi.reshape(ns // 2, 2, DF).transpose(1, 2, 0).reshape(128, ns // 2)
    )  # [128, ns/2]
    m = {"XT": XT, "W1": W1, "W2": W2, "BIN": BIN}
    if has_bout:
        bo = bias_out[ex].astype(np.float32).copy()
        if dummy.any():
            bo[dummy] = 0.0
        m["BOUT"] = np.ascontiguousarray(bo.reshape(ns, 2, 128).transpose(2, 1, 0))
    return m


# ----------------------------------------------------------------- device --


def _build_program(lay, has_bout):
    ns, npad = lay.nslot, lay.npad
    caps, offs = lay.caps, lay.offs
    capmax = int(caps.max())
    ngrp = (ns + GS - 1) // GS
    f32 = mybir.dt.float32

    nc = bacc.Bacc("TRN2", target_bir_lowering=False, debug=False, num_devices=NCORE)
    dXT = nc.dram_tensor("XT", [DM, npad], f32, kind="ExternalInput").ap()
    dW1 = nc.dram_tensor("W1", [128, ns, 2, DF], f32, kind="ExternalInput").ap()
    dW2 = nc.dram_tensor("W2", [128, ns // 2, 2, 128], f32, kind="ExternalInput").ap()
    dBIN = nc.dram_tensor("BIN", [128, ns // 2], f32, kind="ExternalInput").ap()
    if has_bout:
        dBOUT = nc.dram_tensor("BOUT", [128, 2, ns], f32, kind="ExternalInput").ap()
    dOUT = nc.dram_tensor("OUTT", [128, 2, npad], f32, kind="ExternalOutput").ap()

    gelu = mybir.ActivationFunctionType.Gelu

    # XT loaded in two large halves per chunk (big per-partition runs);
    # split at a group boundary so group h's compute only needs half b.
    hgrp = max(1, ngrp // 2)
    hs = min(ns, hgrp * GS)
    mid = int(offs[hs - 1] + caps[hs - 1]) if hs < ns else npad

    with tile.TileContext(nc) as tc:
        with (
            tc.tile_pool(name="wpool", bufs=3) as wpool,
            tc.tile_pool(name="apool", bufs=2) as apool,
            tc.tile_pool(name="cpool", bufs=1) as cpool,
            tc.tile_pool(name="hpool", bufs=4) as hpool,
            tc.tile_pool(name="opool", bufs=3) as opool,
            tc.tile_pool(name="ps1", bufs=2, space="PSUM") as ps1pool,
            tc.tile_pool(name="ps2", bufs=2, space="PSUM") as ps2pool,
        ):
            tBIN = cpool.tile([128, ns // 2], f32, tag="bin")
            nc.sync.dma_start(out=tBIN[:], in_=dBIN[:])
            if has_bout:
                tBOUT = cpool.tile([128, 2, ns], f32, tag="bout")
                nc.sync.dma_start(out=tBOUT[:], in_=dBOUT[:])

            tA = {}  # (chunk, half) -> tile
            halves = [(0, 0, mid)] + ([(1, mid, npad)] if mid < npad else [])

            def load_half(h):
                hb, ha, hbnd = halves[h]
                for ch in range(2):
                    t = apool.tile([128, hbnd - ha], f32, tag=f"a{ch}")
                    nc.sync.dma_start(
                        out=t[:], in_=dXT[ch * 128 : (ch + 1) * 128, ha:hbnd]
                    )
                    tA[(ch, h)] = t

            def a_slice(ch, c0, c1):
                h = 0 if c0 < mid or len(halves) == 1 else 1
                base = halves[h][1]
                return tA[(ch, h)][:, c0 - base : c1 - base]

            load_half(0)

            tWs = []
            for g in range(ngrp):
                s0 = g * GS
                s1 = min(ns, s0 + GS)
                tW1 = wpool.tile([128, s1 - s0, 2, DF], f32, tag="w1")
                nc.sync.dma_start(out=tW1[:], in_=dW1[:, s0:s1, :, :])
                tW2 = wpool.tile([128, (s1 - s0) // 2, 2, 128], f32, tag="w2")
                nc.sync.dma_start(out=tW2[:], in_=dW2[:, s0 // 2 : s1 // 2, :, :])
                tWs.append((tW1, tW2))
                if g == max(0, hgrp - 2) and len(halves) > 1:
                    load_half(1)

                goff = int(offs[s0])
                gw = int(offs[s1 - 1] + caps[s1 - 1]) - goff
                tO = opool.tile([128, 2, gw], f32, tag="ot")

                for p in range((s1 - s0) // 2):
                    se, so = s0 + 2 * p, s0 + 2 * p + 1
                    nE, nO = int(caps[se]), int(caps[so])
                    oE, oO = int(offs[se]), int(offs[so])

                    # even/odd matmul groups run concurrently on disjoint PE
                    # col/row-groups -> they MUST target different PSUM banks
                    # (concurrent drains + whole-bank has_written clears in a
                    # shared bank corrupt/fault).
                    tP1e = ps1pool.tile([128, capmax], f32, tag="ps1e")
                    tP1o = ps1pool.tile([128, capmax], f32, tag="ps1o")
                    tH = hpool.tile([128, capmax], f32, tag="h")

                    for ch in range(2):
                        nc.tensor.matmul(
                            tP1e[0:DF, 0:nE],
                            lhsT=tW1[:, 2 * p, ch, :],
                            rhs=a_slice(ch, oE, oE + nE),
                            start=(ch == 0),
                            stop=(ch == 1),
                        )
                    for ch in range(2):
                        nc.tensor.matmul(
                            tP1o[DF:128, 0:nO],
                            lhsT=tW1[:, 2 * p + 1, ch, :],
                            rhs=a_slice(ch, oO, oO + nO),
                            start=(ch == 0),
                            stop=(ch == 1),
                        )

                    nc.scalar.activation(
                        tH[0:DF, 0:nE],
                        tP1e[0:DF, 0:nE],
                        gelu,
                        bias=tBIN[0:DF, se // 2 : se // 2 + 1],
                    )
                    nc.scalar.activation(
                        tH[DF:128, 0:nO],
                        tP1o[DF:128, 0:nO],
                        gelu,
                        bias=tBIN[DF:128, se // 2 : se // 2 + 1],
                    )

                    tP2e = ps2pool.tile([128, 2, capmax], f32, tag="ps2e")
                    tP2o = ps2pool.tile([128, 2, capmax], f32, tag="ps2o")
                    for ch in range(2):
                        nc.tensor.matmul(
                            tP2e[:, ch, 0:nE],
                            lhsT=tW2[0:DF, p, ch, :],
                            rhs=tH[0:DF, 0:nE],
                            start=True,
                            stop=True,
                        )
                    for ch in range(2):
                        nc.tensor.matmul(
                            tP2o[:, ch, 0:nO],
                            lhsT=tW2[DF:128, p, ch, :],
                            rhs=tH[DF:128, 0:nO],
                            start=True,
                            stop=True,
                        )

                    if has_bout:
                        for ch in range(2):
                            nc.scalar.activation(
                                tO[:, ch, oE - goff : oE - goff + nE],
                                tP2e[:, ch, 0:nE],
                                mybir.ActivationFunctionType.Identity,
                                bias=tBOUT[:, ch, se : se + 1],
                            )
                            nc.scalar.activation(
                                tO[:, ch, oO - goff : oO - goff + nO],
                                tP2o[:, ch, 0:nO],
                                mybir.ActivationFunctionType.Identity,
                                bias=tBOUT[:, ch, so : so + 1],
                            )
                    else:
                        nc.vector.tensor_copy(
                            tO[:, :, oE - goff : oE - goff + nE], tP2e[:, :, 0:nE]
                        )
                        nc.vector.tensor_copy(
                            tO[:, :, oO - goff : oO - goff + nO], tP2o[:, :, 0:nO]
                        )

                nc.sync.dma_start(out=dOUT[:, :, goff : goff + gw], in_=tO[:])

    nc.compile()
    return nc


# ----------------------------------------------------------------- driver --


def _install_profile_shim():
    """antenv.axon_hooks shim: NTFF profiling via libaxon_pjrt exports."""
    import contextlib
    import ctypes
    import sys
    import types

    if "antenv.axon_hooks" in sys.modules:
        return

    lib = ctypes.CDLL("/opt/axon/libaxon_pjrt.so")
    for fn in (lib.axon_start_nrt_profile, lib.axon_stop_nrt_profile):
        fn.restype = ctypes.c_int64
        fn.argtypes = [ctypes.c_char_p, ctypes.c_size_t]

    @contextlib.contextmanager
    def _ntff_profile(output_dir: str, device_ids):
        import jax

        jax.devices()
        d = output_dir.encode()
        rc = lib.axon_start_nrt_profile(d, len(d))
        assert rc == 0, f"axon_start_nrt_profile rc={rc}"
        try:
            yield
        finally:
            lib.axon_stop_nrt_profile(d, len(d))

    mod = types.ModuleType("antenv.axon_hooks")
    mod.get_axon_ntff_profile_hook = lambda: _ntff_profile
    sys.modules["antenv.axon_hooks"] = mod
    import concourse.bass_utils as bu

    bu.upload_artifacts = lambda tmpdir: f"local:{tmpdir}"


def kernel(x, indices, w_in, w_out, bias_in, bias_out):
    global LAST_EXEC_TIME_NS, LAST_TRACE_PATH
    x = np.asarray(x, dtype=np.float32)
    w_in = np.asarray(w_in, dtype=np.float32)
    w_out = np.asarray(w_out, dtype=np.float32)
    bias_in = np.asarray(bias_in, dtype=np.float32)
    bias_out = np.asarray(bias_out, dtype=np.float32)
    e_flat = np.asarray(indices).reshape(-1).astype(np.int64)
    assert x.shape == (B, S, DM) and e_flat.shape == (NPAIR,)

    has_bout = bool(np.any(bias_out))
    lay = _build_layout(e_flat)
    key = (lay.nslot, tuple(int(c) for c in lay.caps), has_bout)
    if key in _CACHE:
        nc = _CACHE[key]
    else:
        nc = _build_program(lay, has_bout)
        _CACHE[key] = nc

    x_flat = x.reshape(NT, DM)
    in_maps = [
        _prep_core_inputs(lay, c, x_flat, w_in, w_out, bias_in, bias_out, has_bout)
        for c in range(NCORE)
    ]

    trace = os.environ.get("KERNEL_TRACE", "") == "1"
    if trace:
        _install_profile_shim()
        res = run_bass_kernel_spmd(
            nc, in_maps, list(range(NCORE)), trace=True,
            trace_cores=[0], stitch_traces=False,
        )
        LAST_EXEC_TIME_NS = res.exec_time_ns
        it = res.instructions_and_trace
        LAST_TRACE_PATH = it[1] if it else None
    else:
        res = run_bass_kernel_spmd(nc, in_maps, list(range(NCORE)))

    out_flat = np.zeros((NPAIR, DM), dtype=np.float32)
    for c in range(NCORE):
        OUTT = res.results[c]["OUTT"]  # [128, 2, npad]
        rows = OUTT.transpose(2, 1, 0).reshape(lay.npad, DM)
        for si in range(lay.nslot):
            pp = lay.slot_pairs[c][si]
            if len(pp):
                o = lay.offs[si]
                out_flat[pp] = rows[o : o + len(pp)]
    return out_flat.reshape(B, S, K, DM)
